# revision 19
# baseline (speedup 1.0000x reference)
"""Trainium2 Bass kernel for nn_Attention_52012053955205.

Multi-head causal attention, B=2 S=2048 D=1024 H=16 HD=64, fp32.

Sharding: 8 cores = 2-way batch x 4-way heads. Each core computes, for its
batch item b and its 4 heads, the partial output sum_h z_h @ W_O_h  as a
full [S, D] tile; the host sums the 4 partials per batch and adds b_O.

Per-core dataflow (scores transposed so the softmax denominator is a free
by-product of matmuls):
  Projections run as fp8(e4m3) DoubleRow matmuls (2 contraction rows per
  partition per cycle, ~3.6x the fp32r FLOP rate): Q/K are single-term
  x8*w8 (weights pre-scaled x64, power-of-2 scales folded into the
  PSUM->SBUF copies; end-to-end ~1e-2 rel err, inside the 2e-2 gate); V is
  3-term error-compensated x8*wv8 + dx8*wv8 + x8*dwv8 (~1e-3 err), where
  dx8/dwv8 are fp8 quantizations of the quantization residuals.
  qt/kt/v/e_t all live in bf16: scores S_T[k,q] = KT_tile.T @ QT_block as
  two K=64 matmuls into one 2-bank PSUM tile; diagonal tiles compute only
  [band:512] (no minimum-N constraint in bf16) and mask just the 128-wide
  diagonal band with a single triangular 0/1 bf16 multiply post-exp.
  One [128, 2, 512-band] exp on ScalarE per (pair, k-tile) writes e_t in
  bf16; a ones column appended to V (V' [s, 65]) makes the z-matmul also
  produce the softmax denominator. z_unnorm^T [65, q] accumulates over
  k-tiles in PSUM; normalization broadcasts 1/denom via K=1 matmuls
  against selector rows, then DVE multiplies.
  Output projection: out[s, D] = znorm_pair^T.T @ W_O_pair (fp32r),
  split into per-512-column units that are DRAINED ONE UNIT PER k-tile
  SLOT inside the next s-block's attention loops (order=4): the exp
  stream paces attention (ACT ~1040ns vs PE ~850ns per slot), so these
  interleaved matmuls keep PE busy during the gaps.
  Emission interleaves proj(sb) / attention(Qb=sb) / out-proj units so
  projection and output-projection PE work fills the ScalarE-bound
  stretches. Measured: 81.5us/rep on HW (baseline 136.8us), rel err 9.8e-3.
"""

import json

import numpy as np

B, S, D, H, HD = 2, 2048, 1024, 16, 64
NCORES = 8
HPC = 4  # heads per core

_STATE = {}


# ---------------------------------------------------------------------------
# Tile tail-drain workaround: walrus in this container rejects >2 sem waits
# on one instruction ("Too many sync wait commands"). Split the tail waits
# across one sync NOP per logical proc; the drain itself then needs none.
# ---------------------------------------------------------------------------
def _patch_tile_drain():
    import concourse.tile as tile
    from concourse.vector_clock import ScopedClock, VectorClock

    if getattr(tile.TileContext, "_drain_split_patch", False):
        return

    def _split_drain_and_barrier(self, tick_clock, wait_clock):
        gc = tick_clock.global_clock
        n = len(gc)
        for proc in range(n):
            t = gc[proc]
            if t > 0:
                vc = VectorClock([t if i == proc else 0 for i in range(n)])
                nop = self.nc.sync.nop(nofuse=True)
                wait_clock.add_sem_waits(nop.ins, ScopedClock({None: vc}))
        self.nc.sync.drain()
        self.nc.all_engine_barrier()
        assert self.sems is not None
        popped = self.nc._tile_sem_poison_stack.pop()
        assert popped is self._sem_poison
        self.nc.clear_and_free_semaphores(list(self.sems.allocated().values()))
        self.nc.all_engine_barrier()

    tile.TileContext._drain_and_barrier = _split_drain_and_barrier
    tile.TileContext._drain_split_patch = True


def _split_waits_bir(bir: bytes) -> bytes:
    """Walrus in this container allows only one sem wait per instruction.
    Spill extra on_wait entries onto same-engine NoOps inserted right
    before the instruction (the NX executes them in stream order)."""
    d = json.loads(bir)
    ctr = 0
    for f in d["functions"]:
        for bb in f["blocks"]:
            new = []
            for ins in bb["instructions"]:
                si = ins.get("sync_info")
                waits = si.get("on_wait", []) if si else []
                if len(waits) > 1:
                    for w in waits[:-1]:
                        ctr += 1
                        new.append(
                            {
                                "debug": ins.get("debug", 0),
                                "engine": ins["engine"],
                                "ins": [],
                                "name": f"I-wsplit-{ctr}",
                                "opcode": "NoOp",
                                "outs": [],
                                "sync_info": {"on_update": [], "on_wait": [w]},
                            }
                        )
                    si["on_wait"] = [waits[-1]]
                new.append(ins)
            bb["instructions"] = new
    return json.dumps(d).encode()


def _hook_wait_split(nc):
    orig = nc.to_json_bytes

    def patched():
        return _split_waits_bir(orig())

    nc.to_json_bytes = patched
    return nc


# ---------------------------------------------------------------------------
# Bass program (identical on all 8 cores; all per-core data arrives as
# ExternalInputs)
# ---------------------------------------------------------------------------
def _build_nc(reps=1, f32r=True, qk_bias=True, upto=3, order=4, zpipe=True, xpf=0, xsplit=4, fp8proj=True, v_bias=False):
    import concourse.bass as bass
    import concourse.mybir as mybir
    import concourse.tile as tile

    FP = mybir.dt.float32
    FR = mybir.dt.float32r
    F8 = mybir.dt.float8e4
    DR = mybir.MatmulPerfMode.DoubleRow
    AF = mybir.ActivationFunctionType
    _patch_tile_drain()

    nc = bass.Bass(target_bir_lowering=False)

    MT = FR if f32r else FP  # dtype for every matmul operand tile

    mm = nc.tensor.matmul

    if fp8proj:
        # fp8 DoubleRow projection operands. Layouts put the contraction
        # dim D on (partition p, chunk-pair cp, subrow i): D = cp*256+i*128+p.
        x8 = nc.dram_tensor("x8", [128, 4, 2, S], F8, kind="ExternalInput")
        dx8 = nc.dram_tensor("dx8", [128, 4, 2, S], F8, kind="ExternalInput")
        wq = nc.dram_tensor("wq", [128, 4, 2, 2, 128], F8, kind="ExternalInput")
        wk = nc.dram_tensor("wk", [128, 4, 2, 2, 128], F8, kind="ExternalInput")
        wv = nc.dram_tensor("wv", [128, 4, 2, 256], F8, kind="ExternalInput")
        dwv = nc.dram_tensor("dwv", [128, 4, 2, 256], F8, kind="ExternalInput")
    else:
        xT = nc.dram_tensor("xt", [D, S], MT, kind="ExternalInput")
        wq = nc.dram_tensor("wq", [2, D, 128], MT, kind="ExternalInput")
        wk = nc.dram_tensor("wk", [2, D, 128], MT, kind="ExternalInput")
        wv = nc.dram_tensor("wv", [D, 256], MT, kind="ExternalInput")
    wo = nc.dram_tensor("wo", [2, 128, D], MT, kind="ExternalInput")
    bq = nc.dram_tensor("bq", [2, 128], FP, kind="ExternalInput")
    bk = nc.dram_tensor("bk", [2, 128], FP, kind="ExternalInput")
    bv = nc.dram_tensor("bv", [256], FP, kind="ExternalInput")
    BF = mybir.dt.bfloat16
    tri = nc.dram_tensor("tri", [128, 128], BF, kind="ExternalInput")
    sel = nc.dram_tensor("sel", [2, 128], MT, kind="ExternalInput")
    out = nc.dram_tensor("out", [S, D], FP, kind="ExternalOutput")

    with tile.TileContext(nc) as tc:
        with (
            nc.allow_low_precision(reason="tf32 (fp32r) matmul pipeline"),
            tc.tile_pool(name="consts", bufs=1) as consts,
            tc.tile_pool(name="xp", bufs=3) as xp,
            tc.tile_pool(name="qk", bufs=1) as qk,
            tc.tile_pool(name="vp", bufs=1) as vp,
            tc.tile_pool(name="zp", bufs=1) as zp,
            tc.tile_pool(name="etp", bufs=5) as etp,
            tc.tile_pool(name="bcp", bufs=2) as bcp,
            tc.tile_pool(name="rdpool", bufs=4) as rdpool,
            tc.tile_pool(name="ostp", bufs=2) as ostp,
            tc.tile_pool(name="psA", bufs=2, space="PSUM") as psA,
            tc.tile_pool(name="psB", bufs=4, space="PSUM") as psB,
        ):
            # ---- constants ----
            # group A: needed by the first projections -- DMA'd first so the
            # startup x block isn't bandwidth-starved by cold constants
            if fp8proj:
                wq_sb = consts.tile([128, 4, 2, 2, 128], F8, tag="wq")
                nc.sync.dma_start(out=wq_sb, in_=wq[:])
                x0_t = xp.tile([128, 4, 2, 512], F8, tag="x", name="x_pre0")
                dx0_t = xp.tile([128, 4, 2, 512], F8, tag="dx", name="dx_pre0")
                nc.sync.dma_start(out=x0_t, in_=x8[:, :, :, 0:512])
                nc.sync.dma_start(out=dx0_t, in_=dx8[:, :, :, 0:512])
                wk_sb = consts.tile([128, 4, 2, 2, 128], F8, tag="wk")
                nc.sync.dma_start(out=wk_sb, in_=wk[:])
                wv_sb = consts.tile([128, 4, 2, 256], F8, tag="wv")
                nc.sync.dma_start(out=wv_sb, in_=wv[:])
                dwv_sb = consts.tile([128, 4, 2, 256], F8, tag="dwv")
                nc.sync.dma_start(out=dwv_sb, in_=dwv[:])
            else:
                xTr = xT[:].rearrange("(c p) s -> p c s", p=128)
                wq_sb = consts.tile([128, 2, 8, 128], MT, tag="wq")
                nc.sync.dma_start(
                    out=wq_sb, in_=wq[:].rearrange("a (c p) d -> p a c d", p=128)
                )
                x0_t = xp.tile([128, 8, 512], MT, tag="x", name="x_pre0")
                for dd in range(4):
                    nc.sync.dma_start(
                        out=x0_t[:, 2 * dd : 2 * dd + 2, :],
                        in_=xTr[:, 2 * dd : 2 * dd + 2, 0:512],
                    )
                wk_sb = consts.tile([128, 2, 8, 128], MT, tag="wk")
                nc.sync.dma_start(
                    out=wk_sb, in_=wk[:].rearrange("a (c p) d -> p a c d", p=128)
                )
                wv_sb = consts.tile([128, 8, 256], MT, tag="wv")
                nc.sync.dma_start(
                    out=wv_sb, in_=wv[:].rearrange("(c p) d -> p c d", p=128)
                )
            if qk_bias:
                bq_sb = consts.tile([128, 2], FP, tag="bq")
                nc.sync.dma_start(out=bq_sb, in_=bq[:].rearrange("a p -> p a"))
                bk_sb = consts.tile([128, 2], FP, tag="bk")
                nc.sync.dma_start(out=bk_sb, in_=bk[:].rearrange("a p -> p a"))
            bvbc_sb = consts.tile([128, 4, 64], FP, tag="bvbc")
            nc.sync.dma_start(
                out=bvbc_sb,
                in_=bass.AP(tensor=bv, offset=0, ap=[[0, 128], [1, 256]]),
            )
            # group B: not needed until attention / out-proj of the first
            # s-block -- emitted lazily below
            wo_sb = consts.tile([128, 2, D], MT, tag="wo")
            tri_sb = consts.tile([128, 128], BF, tag="tri")
            sel_sb = consts.tile([1, 2, 128], MT, tag="sel")

            def emit_const_group_b():
                nc.sync.dma_start(out=tri_sb, in_=tri[:])
                nc.sync.dma_start(
                    out=sel_sb,
                    in_=bass.AP(
                        tensor=sel, offset=0, ap=[[256, 1], [128, 2], [1, 128]]
                    ),
                )
                nc.sync.dma_start(
                    out=wo_sb, in_=wo[:].rearrange("a p d -> p a d")
                )

            def emit_x(sb, x_pre=None):
                if x_pre is not None:
                    return x_pre
                if fp8proj:
                    x_t = xp.tile([128, 4, 2, 512], F8, tag="x", name=f"x_{sb}")
                    dx_t = xp.tile([128, 4, 2, 512], F8, tag="dx", name=f"dx_{sb}")
                    for dd in range(2):
                        nc.sync.dma_start(
                            out=x_t[:, 2 * dd : 2 * dd + 2, :, :],
                            in_=x8[:, 2 * dd : 2 * dd + 2, :, sb * 512 : (sb + 1) * 512],
                        )
                        nc.sync.dma_start(
                            out=dx_t[:, 2 * dd : 2 * dd + 2, :, :],
                            in_=dx8[:, 2 * dd : 2 * dd + 2, :, sb * 512 : (sb + 1) * 512],
                        )
                    return (x_t, dx_t)
                x_t = xp.tile([128, 8, 512], MT, tag="x", name=f"x_{sb}")
                w = 8 // xsplit
                for dd in range(xsplit):
                    nc.sync.dma_start(
                        out=x_t[:, w * dd : w * dd + w, :],
                        in_=xTr[
                            :, w * dd : w * dd + w, sb * 512 : (sb + 1) * 512
                        ],
                    )
                return x_t

            # scale folding for the fp8 path: weights host-quantized at x64,
            # score 1/sqrt(HD)=1/8 folded into the Q copy
            SQ, SK, SV = 2.0**-9, 2.0**-6, 2.0**-6

            def emit_qk(sb, x_t, qt_sb, kt_sb, pairs=(0, 1)):
                if fp8proj:
                    x_t = x_t[0]
                for pair in pairs:
                    psQ = psA.tile([128, 2, 512], FP, tag="A", name=f"psQ_{sb}_{pair}")
                    if fp8proj:
                        for c in range(4):
                            mm(
                                psQ[:, 0, :],
                                wq_sb[:, c, :, pair, :],
                                x_t[:, c, :, :],
                                start=(c == 0),
                                stop=(c == 3),
                                perf_mode=DR,
                            )
                        for c in range(4):
                            mm(
                                psQ[:, 1, :],
                                wk_sb[:, c, :, pair, :],
                                x_t[:, c, :, :],
                                start=(c == 0),
                                stop=(c == 3),
                                perf_mode=DR,
                            )
                    else:
                        for c in range(8):
                            mm(
                                psQ[:, 0, :],
                                wq_sb[:, pair, c, :],
                                x_t[:, c, :],
                                start=(c == 0),
                                stop=(c == 7),
                            )
                        for c in range(8):
                            mm(
                                psQ[:, 1, :],
                                wk_sb[:, pair, c, :],
                                x_t[:, c, :],
                                start=(c == 0),
                                stop=(c == 7),
                            )
                    qt_dst = qt_sb[:, pair, sb * 512 : (sb + 1) * 512]
                    kt_dst = kt_sb[:, pair, sb * 512 : (sb + 1) * 512]
                    sq = SQ if fp8proj else 1.0
                    sk = SK if fp8proj else 1.0
                    if qk_bias:
                        nc.scalar.activation(
                            qt_dst, psQ[:, 0, :], AF.Identity,
                            bias=bq_sb[:, pair : pair + 1], scale=sq,
                        )
                        nc.scalar.activation(
                            kt_dst, psQ[:, 1, :], AF.Identity,
                            bias=bk_sb[:, pair : pair + 1], scale=sk,
                        )
                    elif fp8proj:
                        if pair == 0:
                            nc.vector.tensor_scalar_mul(qt_dst, psQ[:, 0, :], sq)
                            nc.vector.tensor_scalar_mul(kt_dst, psQ[:, 1, :], sk)
                        else:
                            nc.scalar.activation(qt_dst, psQ[:, 0, :], AF.Copy, scale=sq)
                            nc.scalar.activation(kt_dst, psQ[:, 1, :], AF.Copy, scale=sk)
                    elif pair == 0:
                        nc.vector.tensor_copy(qt_dst, psQ[:, 0, :])
                        nc.vector.tensor_copy(kt_dst, psQ[:, 1, :])
                    else:
                        nc.scalar.activation(qt_dst, psQ[:, 0, :], AF.Copy)
                        nc.scalar.activation(kt_dst, psQ[:, 1, :], AF.Copy)

            def emit_v(sb, x_t, v_sb):
                if fp8proj:
                    x_t, dx_t = x_t
                for stl in range(4):
                    st = sb * 4 + stl
                    psV = psB.tile([128, 256], FP, tag="ZB", name=f"psV_{st}")
                    if fp8proj:
                        ops = [(x_t, wv_sb), (dx_t, wv_sb), (x_t, dwv_sb)]
                        for t, (xx, ww) in enumerate(ops):
                            for c in range(4):
                                mm(
                                    psV,
                                    xx[:, c, :, stl * 128 : (stl + 1) * 128],
                                    ww[:, c, :, :],
                                    start=(t == 0 and c == 0),
                                    stop=(t == 2 and c == 3),
                                    perf_mode=DR,
                                )
                        nc.vector.tensor_scalar_mul(
                            v_sb[:, st, :, 0:64],
                            psV.rearrange("p (h d) -> p h d", h=4),
                            SV,
                        )
                        if v_bias:
                            nc.vector.tensor_add(
                                v_sb[:, st, :, 0:64],
                                v_sb[:, st, :, 0:64],
                                bvbc_sb,
                            )
                    else:
                        for c in range(8):
                            mm(
                                psV,
                                x_t[:, c, stl * 128 : (stl + 1) * 128],
                                wv_sb[:, c, :],
                                start=(c == 0),
                                stop=(c == 7),
                            )
                        nc.vector.tensor_add(
                            v_sb[:, st, :, 0:64],
                            psV.rearrange("p (h d) -> p h d", h=4),
                            bvbc_sb,
                        )

            def emit_attn_both(Qb, qt_sb, kt_sb, v_sb, znp):
                """Both head pairs' attention for one q-block, kt loops
                interleaved: two independent score->exp->z chains keep PE
                fed while either waits on ScalarE."""
                q0, q1 = Qb * 512, (Qb + 1) * 512
                ktmax = 4 * (Qb + 1)
                psZs = {}
                for pair in range(2):
                    for hh in range(2):
                        psZs[(pair, hh)] = psB.tile(
                            [65, 512], FP, tag="ZB",
                            name=f"psZ_{pair}_{Qb}_{hh}",
                        )
                for kt in range(ktmax):
                    diag = kt >= 4 * Qb
                    r = (kt - 4 * Qb) * 128 if diag else 0
                    for pair in range(2):
                        psS = psA.tile(
                            [128, 2, 512], FP, tag="A",
                            name=f"psS_{pair}_{Qb}_{kt}",
                        )
                        for hh in range(2):
                            po = hh * 64
                            mm(
                                psS[:, hh, r:512],
                                kt_sb[po : po + 64, pair, kt * 128 : (kt + 1) * 128],
                                qt_sb[po : po + 64, pair, q0 + r : q1],
                                start=True,
                                stop=True,
                            )
                        e_t = etp.tile(
                            [128, 2, 512], BF, tag="et",
                            name=f"et_{pair}_{Qb}_{kt}",
                        )
                        nc.scalar.activation(
                            e_t[:, :, r:512], psS[:, :, r:512], AF.Exp
                        )
                        if diag:
                            for hh in range(2):
                                nc.vector.tensor_mul(
                                    e_t[:, hh, r : r + 128],
                                    e_t[:, hh, r : r + 128],
                                    tri_sb,
                                )
                        for hh in range(2):
                            mm(
                                psZs[(pair, hh)][:, r:512],
                                v_sb[:, kt, 2 * pair + hh, :],
                                e_t[:, hh, r:512],
                                start=(kt == 0),
                                stop=(kt == ktmax - 1),
                            )
                for pair in range(2):
                    rds = []
                    for hh in range(2):
                        rd_h = rdpool.tile(
                            [1, 512], MT, tag="rd", name=f"rd_{pair}_{Qb}_{hh}"
                        )
                        rds.append(rd_h)
                        nc.vector.reciprocal(rd_h, psZs[(pair, hh)][64:65, :])
                    bc = psA.tile([128, 512], FP, tag="A", name=f"bc_{pair}_{Qb}")
                    mm(bc, sel_sb[:, 0, :], rds[0], start=True, stop=False)
                    mm(bc, sel_sb[:, 1, :], rds[1], start=False, stop=True)
                    bcs = bcp.tile(
                        [128, 512], FP, tag="bcs", name=f"bcs_{pair}_{Qb}"
                    )
                    nc.vector.tensor_copy(bcs, bc)
                    nc.vector.tensor_mul(
                        znp[0:64, pair, Qb, :],
                        psZs[(pair, 0)][0:64, :],
                        bcs[0:64, :],
                    )
                    zc = bcp.tile([128, 512], FP, tag="zc", name=f"zc_{pair}_{Qb}")
                    nc.vector.tensor_copy(zc[64:128, :], psZs[(pair, 1)][0:64, :])
                    nc.vector.tensor_mul(
                        znp[64:128, pair, Qb, :],
                        zc[64:128, :],
                        bcs[64:128, :],
                    )

            def emit_attn(pair, Qb, qt_sb, kt_sb, v_sb, znp, filler=None):
                """Attention for one head pair and one 512-wide q-block."""
                q0, q1 = Qb * 512, (Qb + 1) * 512
                ktmax = 4 * (Qb + 1)
                psZs = []
                for hh in range(2):
                    psZ_h = psB.tile(
                        [65, 512], FP, tag="ZB", name=f"psZ_{pair}_{Qb}_{hh}"
                    )
                    psZs.append(psZ_h)
                def emit_z(kt, e_t, r):
                    for hh in range(2):
                        mm(
                            psZs[hh][:, r:512],
                            v_sb[:, kt, 2 * pair + hh, :],
                            e_t[:, hh, r:512],
                            start=(kt == 0),
                            stop=(kt == ktmax - 1),
                        )

                pending = None  # (kt, e_t, r) -- z emitted one kt behind
                for kt in range(ktmax):
                    # diagonal k-tiles: q-columns < r are fully masked, so
                    # scores/exp/z are all computed on [r:512] only (bf16
                    # matmuls have no minimum-N rate constraint)
                    diag = kt >= 4 * Qb
                    r = (kt - 4 * Qb) * 128 if diag else 0
                    # both heads' scores in one 2-bank PSUM tile; the two
                    # K=64 matmuls hit disjoint PE row groups and overlap
                    psS = psA.tile(
                        [128, 2, 512], FP, tag="A", name=f"psS_{pair}_{Qb}_{kt}"
                    )
                    for hh in range(2):
                        po = hh * 64
                        mm(
                            psS[:, hh, r:512],
                            kt_sb[po : po + 64, pair, kt * 128 : (kt + 1) * 128],
                            qt_sb[po : po + 64, pair, q0 + r : q1],
                            start=True,
                            stop=True,
                        )
                    e_t = etp.tile(
                        [128, 2, 512], BF, tag="et", name=f"et_{pair}_{Qb}_{kt}"
                    )
                    nc.scalar.activation(
                        e_t[:, :, r:512], psS[:, :, r:512], AF.Exp
                    )
                    if diag:
                        # causal 0/1 mask: only the 128-wide diagonal band
                        # needs it; columns past the band are unmasked
                        for hh in range(2):
                            nc.vector.tensor_mul(
                                e_t[:, hh, r : r + 128],
                                e_t[:, hh, r : r + 128],
                                tri_sb,
                            )
                    if not zpipe:
                        emit_z(kt, e_t, r)
                    else:
                        if pending is not None:
                            emit_z(*pending)
                        pending = (kt, e_t, r)
                    if filler is not None:
                        filler()
                if zpipe:
                    emit_z(*pending)
                rds = []
                for hh in range(2):
                    rd_h = rdpool.tile(
                        [1, 512], MT, tag="rd", name=f"rd_{pair}_{Qb}_{hh}"
                    )
                    rds.append(rd_h)
                    nc.vector.reciprocal(rd_h, psZs[hh][64:65, :])
                # broadcast 1/denom of both heads to a stacked [128, 512]
                # tile via two K=1 matmuls against selector rows
                bc = psB.tile([128, 512], FP, tag="ZB", name=f"bc_{pair}_{Qb}")
                mm(bc, sel_sb[:, 0, :], rds[0], start=True, stop=False)
                mm(bc, sel_sb[:, 1, :], rds[1], start=False, stop=True)
                bcs = bcp.tile([128, 512], FP, tag="bcs", name=f"bcs_{pair}_{Qb}")
                nc.vector.tensor_copy(bcs, bc)
                # hh=0: partitions already 0..63 everywhere
                nc.vector.tensor_mul(
                    znp[0:64, pair, Qb, :],
                    psZs[0][0:64, :],
                    bcs[0:64, :],
                )
                # hh=1: single-src shift-copy 0..63 -> 64..127, then mul
                zc = bcp.tile([128, 512], FP, tag="zc", name=f"zc_{pair}_{Qb}")
                nc.vector.tensor_copy(zc[64:128, :], psZs[1][0:64, :])
                nc.vector.tensor_mul(
                    znp[64:128, pair, Qb, :],
                    zc[64:128, :],
                    bcs[64:128, :],
                )

            ost_tiles = {}

            def emit_out_unit(st, Db, znp):
                """One Db half of the output projection for one s-tile."""
                Qb, soff = st // 4, (st % 4) * 128
                if Db == 0:
                    ost_tiles[st] = ostp.tile(
                        [128, D], FP, tag="ost", name=f"ost_{st}"
                    )
                ost_t = ost_tiles[st]
                if True:
                    psO = psB.tile(
                        [128, 512], FP, tag="ZB", name=f"psO_{st}_{Db}"
                    )
                    for pair in range(2):
                        mm(
                            psO,
                            znp[:, pair, Qb, soff : soff + 128],
                            wo_sb[:, pair, Db * 512 : (Db + 1) * 512],
                            start=(pair == 0),
                            stop=(pair == 1),
                        )
                    if Db == 0:
                        nc.vector.tensor_copy(
                            ost_t[:, Db * 512 : (Db + 1) * 512], psO
                        )
                    else:
                        nc.scalar.activation(
                            ost_t[:, Db * 512 : (Db + 1) * 512], psO, AF.Copy
                        )
                if Db == 1:
                    # issue from the DVE sequencer: the store's wait target
                    # is the DVE copy that just ran there, so it can't
                    # head-of-line block the SP stream that issues x loads
                    nc.sync.dma_start(
                        out=out[st * 128 : (st + 1) * 128, :], in_=ost_t
                    )
                    del ost_tiles[st]

            def emit_out(st, znp):
                for Db in range(2):
                    emit_out_unit(st, Db, znp)

            for _rep in range(reps):
                qt_sb = qk.tile([128, 2, S], BF, tag="qt")
                kt_sb = qk.tile([128, 2, S], BF, tag="kt")
                v_sb = vp.tile([128, 16, 4, 65], BF, tag="v")
                znp = zp.tile([128, 2, 4, 512], MT, tag="zn")
                # ones column of V' (written once; proj fills the rest)
                nc.vector.memset(v_sb[:, :, :, 64:65], 1.0)

                # interleaved emission: attention for q-block Qb only needs
                # projections of s-blocks <= Qb, so proj(sb) / attn(Qb=sb) /
                # out-proj(Qb=sb) alternate -- projection PE work fills the
                # gaps while ScalarE grinds through the exps
                x_tiles = {}
                x_pre0 = (x0_t, dx0_t) if fp8proj else x0_t
                for sb in range(4):
                    if sb not in x_tiles:
                        x_tiles[sb] = emit_x(
                            sb, x_pre=x_pre0 if (_rep == 0 and sb == 0) else None
                        )
                    x_t = x_tiles[sb]
                    # prefetch x blocks ahead
                    for ahead in range(1, xpf + 1):
                        if sb + ahead <= 3 and sb + ahead not in x_tiles:
                            x_tiles[sb + ahead] = emit_x(sb + ahead)
                    if order == 4:
                        # out-proj of the previous s-block drained one Db-unit
                        # at a time between attention kt slots: PE filler for
                        # the ACT-paced exp stream
                        emit_qk(sb, x_t, qt_sb, kt_sb)
                        emit_v(sb, x_t, v_sb)
                        if _rep == 0 and sb == 0:
                            emit_const_group_b()
                        pend = (
                            [(st, Db) for st in range(4 * sb - 4, 4 * sb)
                             for Db in range(2)]
                            if sb > 0 else []
                        )
                        slots = [2 * 4 * (sb + 1)]

                        def filler(pend=pend, slots=slots):
                            slots[0] -= 1
                            if not pend:
                                return
                            k = max(1, -(-len(pend) // max(1, slots[0] + 1)))
                            for _ in range(min(k, len(pend))):
                                st, Db = pend.pop(0)
                                emit_out_unit(st, Db, znp)

                        if upto >= 2:
                            emit_attn(0, sb, qt_sb, kt_sb, v_sb, znp, filler)
                            emit_attn(1, sb, qt_sb, kt_sb, v_sb, znp, filler)
                        for st, Db in pend:
                            emit_out_unit(st, Db, znp)
                        if sb == 3:
                            for st in range(12, 16):
                                emit_out(st, znp)
                    elif order == 3:
                        emit_qk(sb, x_t, qt_sb, kt_sb)
                        emit_v(sb, x_t, v_sb)
                        if _rep == 0 and sb == 0:
                            emit_const_group_b()
                        if upto >= 2:
                            emit_attn_both(sb, qt_sb, kt_sb, v_sb, znp)
                    elif order == 0:
                        emit_qk(sb, x_t, qt_sb, kt_sb)
                        emit_v(sb, x_t, v_sb)
                        if _rep == 0 and sb == 0:
                            emit_const_group_b()
                        if upto >= 2:
                            for pair in range(2):
                                emit_attn(pair, sb, qt_sb, kt_sb, v_sb, znp)
                    elif order == 2:
                        # out-proj of the previous s-block emitted between
                        # the two attention passes as mid-segment PE filler
                        emit_qk(sb, x_t, qt_sb, kt_sb)
                        emit_v(sb, x_t, v_sb)
                        if _rep == 0 and sb == 0:
                            emit_const_group_b()
                        if upto >= 2:
                            emit_attn(0, sb, qt_sb, kt_sb, v_sb, znp)
                        if upto >= 3 and sb > 0:
                            for st in range(4 * sb - 4, 4 * sb):
                                emit_out(st, znp)
                        if upto >= 2:
                            emit_attn(1, sb, qt_sb, kt_sb, v_sb, znp)
                    else:
                        # pair-1 projections emitted between the two
                        # attention passes: PE fills attention(pair0)'s
                        # ScalarE-bound stretch with projection matmuls
                        emit_qk(sb, x_t, qt_sb, kt_sb, pairs=(0,))
                        emit_v(sb, x_t, v_sb)
                        if _rep == 0 and sb == 0:
                            emit_const_group_b()
                        if upto >= 2:
                            emit_attn(0, sb, qt_sb, kt_sb, v_sb, znp)
                        emit_qk(sb, x_t, qt_sb, kt_sb, pairs=(1,))
                        if upto >= 2:
                            emit_attn(1, sb, qt_sb, kt_sb, v_sb, znp)
                    if upto >= 3 and (order not in (2, 4) or sb == 3):
                        if order == 4:
                            continue
                        for st in range(4 * sb, 4 * sb + 4):
                            emit_out(st, znp)

    return _hook_wait_split(nc)



# ---------------------------------------------------------------------------
# Persistent PJRT runner (mirrors run_bass_via_pjrt, but keeps the jitted
# callable so repeated kernel() calls don't recompile)
# ---------------------------------------------------------------------------
class _Runner:
    def __init__(self, nc):
        import jax
        import jax.numpy as jnp  # noqa: F401
        import numpy as _np
        from jax.experimental.shard_map import shard_map
        from jax.sharding import Mesh, PartitionSpec
        import concourse.mybir as mybir
        from concourse.bass2jax import (
            _bass_exec_p,
            install_neuronx_cc_hook,
            partition_id_tensor,
        )

        install_neuronx_cc_hook()
        self.jax = jax
        pname = nc.partition_id_tensor.name if nc.partition_id_tensor else None
        in_names, out_names, out_avals, zero_outs = [], [], [], []
        for alloc in nc.m.functions[0].allocations:
            if not isinstance(alloc, mybir.MemoryLocationSet):
                continue
            name = alloc.memorylocations[0].name
            if alloc.kind == "ExternalInput":
                if name == pname:
                    continue
                in_names.append(name)
            elif alloc.kind == "ExternalOutput":
                shape = tuple(alloc.tensor_shape)
                dtype = mybir.dt.np(alloc.dtype)
                out_names.append(name)
                out_avals.append(jax.core.ShapedArray(shape, dtype))
                zero_outs.append(_np.zeros(shape, dtype))
        self.in_names, self.out_names = list(in_names), list(out_names)
        self.out_avals, self.zero_outs = out_avals, zero_outs
        n_params, n_outs = len(in_names), len(out_names)
        self.n_params = n_params
        all_names = in_names + out_names
        if pname is not None:
            all_names = all_names + [pname]

        def _body(*args):
            operands = list(args)
            if pname is not None:
                operands.append(partition_id_tensor())
            outs = _bass_exec_p.bind(
                *operands,
                out_avals=tuple(out_avals),
                in_names=tuple(all_names),
                out_names=tuple(out_names),
                lowering_input_output_aliases=(),
                sim_require_finite=True,
                sim_require_nnan=True,
                nc=nc,
            )
            return tuple(outs)

        devices = jax.devices()[:NCORES]
        mesh = Mesh(np.asarray(devices), ("core",))
        in_specs = (PartitionSpec("core"),) * (n_params + n_outs)
        out_specs = (PartitionSpec("core"),) * n_outs
        self.fn = jax.jit(
            shard_map(
                _body,
                mesh=mesh,
                in_specs=in_specs,
                out_specs=out_specs,
                check_rep=False,
            ),
            donate_argnums=tuple(range(n_params, n_params + n_outs)),
            keep_unused=True,
        )

    def device_put_inputs(self, concat_in):
        return [self.jax.device_put(a) for a in concat_in]

    def time_exec(self, dev_in, iters=8):
        """Repeat execution with device-resident inputs; the previous call's
        (donated, fully-overwritten) outputs serve as the next call's output
        buffers, so nothing moves over the axon tunnel."""
        import time as _time

        zeros = [
            np.zeros((NCORES * z.shape[0], *z.shape[1:]), z.dtype)
            for z in self.zero_outs
        ]
        r = self.fn(*dev_in, *zeros)
        self.jax.block_until_ready(r)
        times = []
        for _ in range(iters):
            t0 = _time.perf_counter()
            r = self.fn(*dev_in, *r)
            self.jax.block_until_ready(r)
            times.append(_time.perf_counter() - t0)
        return times

    def concat_inputs(self, in_maps):
        return [
            np.concatenate([in_maps[c][n] for c in range(NCORES)], axis=0)
            for n in self.in_names
        ]

    def run_concat(self, concat_in):
        zeros = [
            np.zeros((NCORES * z.shape[0], *z.shape[1:]), z.dtype)
            for z in self.zero_outs
        ]
        outs = self.fn(*concat_in, *zeros)
        outs = [np.asarray(o) for o in outs]
        return outs

    def run(self, in_maps):
        outs = self.run_concat(self.concat_inputs(in_maps))
        per_core = []
        for c in range(NCORES):
            m = {}
            for i, n in enumerate(self.out_names):
                shp = self.out_avals[i].shape
                m[n] = outs[i].reshape(NCORES, *shp)[c]
            per_core.append(m)
        return per_core


def _round_tf32(a):
    """Round fp32 -> TF32 (10-bit mantissa, RNE) so device-side fp32r
    consumers see pre-rounded values."""
    u = np.ascontiguousarray(a, dtype=np.float32).view(np.uint32)
    r = (u + np.uint32(0x1000) + ((u >> np.uint32(13)) & np.uint32(1))) & np.uint32(0xFFFFE000)
    return r.view(np.float32)


def _make_tri():
    """Within-band causal 0/1 mask: keep k-row p for band column j iff p<=j."""
    import ml_dtypes

    p = np.arange(128)[:, None]
    j = np.arange(128)[None, :]
    return (p <= j).astype(ml_dtypes.bfloat16)


def _q8(a):
    import ml_dtypes

    return np.asarray(a, dtype=np.float32).astype(ml_dtypes.float8_e4m3)


def _pack_d(a):
    """[D, ...] -> [128, 4, 2, ...] with D = cp*256 + i*128 + p."""
    rest = a.shape[1:]
    return np.ascontiguousarray(
        a.reshape(4, 2, 128, *rest).transpose(2, 0, 1, *range(3, 3 + len(rest)))
    )


def _prep_core_inputs(inputs, fp8proj=True):
    """Shard + repack the full problem inputs into per-core input maps."""
    x = np.asarray(inputs["normalized_resid_pre"], dtype=np.float32)
    W_Q = np.asarray(inputs["W_Q"], dtype=np.float32)
    W_K = np.asarray(inputs["W_K"], dtype=np.float32)
    W_V = np.asarray(inputs["W_V"], dtype=np.float32)
    W_O = np.asarray(inputs["W_O"], dtype=np.float32)
    b_Q = np.asarray(inputs["b_Q"], dtype=np.float32)
    b_K = np.asarray(inputs["b_K"], dtype=np.float32)
    b_V = np.asarray(inputs["b_V"], dtype=np.float32)

    scale = np.float32(1.0 / np.sqrt(HD))
    SW = np.float32(64.0)  # fp8 weight pre-scale (power of 2)
    tri = _make_tri()
    sel = np.zeros((2, 128), dtype=np.float32)
    sel[0, 0:64] = 1.0
    sel[1, 64:128] = 1.0

    in_maps = []
    for c in range(NCORES):
        b, g = c // 4, c % 4
        hs = [4 * g + i for i in range(HPC)]
        wo_p = np.zeros((2, 128, D), dtype=np.float32)
        bq_p = np.zeros((2, 128), dtype=np.float32)
        bk_p = np.zeros((2, 128), dtype=np.float32)
        wq_p = np.zeros((2, D, 128), dtype=np.float32)
        wk_p = np.zeros((2, D, 128), dtype=np.float32)
        for pr in range(2):
            h0, h1 = hs[2 * pr], hs[2 * pr + 1]
            qsc = 1.0 if fp8proj else scale
            wq_p[pr, :, 0:64] = W_Q[h0] * qsc
            wq_p[pr, :, 64:128] = W_Q[h1] * qsc
            wk_p[pr, :, 0:64] = W_K[h0]
            wk_p[pr, :, 64:128] = W_K[h1]
            wo_p[pr, 0:64, :] = W_O[h0]
            wo_p[pr, 64:128, :] = W_O[h1]
            bq_p[pr, 0:64] = b_Q[h0] * scale
            bq_p[pr, 64:128] = b_Q[h1] * scale
            bk_p[pr, 0:64] = b_K[h0]
            bk_p[pr, 64:128] = b_K[h1]
        wv_p = np.concatenate([W_V[h] for h in hs], axis=1)  # [D, 256]
        bv_p = np.concatenate([b_V[h] for h in hs], axis=0)  # [256]
        m = {
            "wo": _round_tf32(wo_p),
            "bq": bq_p,
            "bk": bk_p,
            "bv": np.ascontiguousarray(bv_p),
            "tri": tri,
            "sel": sel,
        }
        if fp8proj:
            xT = np.ascontiguousarray(x[b].T)  # [D, S]
            x8 = _q8(xT)
            dx8 = _q8(xT - x8.astype(np.float32))
            wq8 = _q8(wq_p.transpose(1, 0, 2) * SW)  # [D, 2, 128]
            wk8 = _q8(wk_p.transpose(1, 0, 2) * SW)
            wv16 = wv_p * SW
            wv8 = _q8(wv16)
            dwv8 = _q8(wv16 - wv8.astype(np.float32))
            m.update(
                x8=_pack_d(x8),
                dx8=_pack_d(dx8),
                wq=_pack_d(wq8),
                wk=_pack_d(wk8),
                wv=_pack_d(wv8),
                dwv=_pack_d(dwv8),
            )
        else:
            m.update(
                xt=_round_tf32(np.ascontiguousarray(x[b].T)),
                wq=_round_tf32(wq_p),
                wk=_round_tf32(wk_p),
                wv=np.ascontiguousarray(_round_tf32(wv_p)),
            )
        in_maps.append(m)
    return in_maps


def _get_state(qk_bias=True, v_bias=False):
    key = (qk_bias, v_bias)
    if key not in _STATE:
        _STATE[key] = _Runner(_build_nc(qk_bias=qk_bias, v_bias=v_bias))
    return _STATE[key]


def kernel(**inputs):
    need_qk_bias = bool(
        np.any(np.asarray(inputs["b_Q"])) or np.any(np.asarray(inputs["b_K"]))
    )
    need_v_bias = bool(np.any(np.asarray(inputs["b_V"])))
    st = _get_state(qk_bias=need_qk_bias, v_bias=need_v_bias)
    in_maps = _prep_core_inputs(inputs)
    per_core = st.run(in_maps)
    b_O = np.asarray(inputs["b_O"], dtype=np.float32)
    out = np.zeros((B, S, D), dtype=np.float32)
    for c in range(NCORES):
        out[c // 4] += per_core[c]["out"]
    out += b_O[None, None, :]
    return out



# revision 29
# speedup vs baseline: 1.2685x; 1.2685x over previous
"""Trainium2 Bass kernel for nn_Attention_52012053955205.

Multi-head causal attention, B=2 S=2048 D=1024 H=16 HD=64, fp32.

Sharding: 8 cores = 2-way batch x 4-way heads. Each core computes, for its
batch item b and its 4 heads, the partial output sum_h z_h @ W_O_h  as a
full [S, D] tile; the host sums the 4 partials per batch and adds b_O.

Per-core dataflow (scores transposed so the softmax denominator is a free
by-product of matmuls):
  Projections run as fp8(e4m3) DoubleRow matmuls (2 contraction rows per
  partition per cycle, ~3.6x the fp32r FLOP rate): Q/K are single-term
  x8*w8 (weights pre-scaled x64, power-of-2 scales folded into the
  PSUM->SBUF copies; end-to-end ~1e-2 rel err, inside the 2e-2 gate); V is
  3-term error-compensated x8*wv8 + dx8*wv8 + x8*dwv8 (~1e-3 err), where
  dx8/dwv8 are fp8 quantizations of the quantization residuals.
  qt/kt/v/e_t all live in bf16: scores S_T[k,q] = KT_tile.T @ QT_block as
  two K=64 matmuls into one 2-bank PSUM tile; diagonal tiles compute only
  [band:512] (no minimum-N constraint in bf16) and mask just the 128-wide
  diagonal band with a single triangular 0/1 bf16 multiply post-exp.
  One [128, 2, 512-band] exp on ScalarE per (pair, k-tile) writes e_t in
  bf16; a ones column appended to V (V' [s, 65]) makes the z-matmul also
  produce the softmax denominator. z_unnorm^T [65, q] accumulates over
  k-tiles in PSUM; normalization broadcasts 1/denom via K=1 matmuls
  against selector rows, then DVE multiplies.
  Output projection: out[s, D] = znorm_pair^T.T @ W_O_pair (fp32r),
  split into per-512-column units that are DRAINED ONE UNIT PER k-tile
  SLOT inside the next s-block's attention loops (order=4): the exp
  stream paces attention (ACT ~1040ns vs PE ~850ns per slot), so these
  interleaved matmuls keep PE busy during the gaps.
  Emission interleaves proj(sb) / attention(Qb=sb) / out-proj units so
  projection and output-projection PE work fills the ScalarE-bound
  stretches. Measured: 81.5us/rep on HW (baseline 136.8us), rel err 9.8e-3.
"""

import json

import numpy as np

B, S, D, H, HD = 2, 2048, 1024, 16, 64
NCORES = 8
HPC = 4  # heads per core
ZT = True  # transposed-z dataflow (order=5)

_STATE = {}


# ---------------------------------------------------------------------------
# Tile tail-drain workaround: walrus in this container rejects >2 sem waits
# on one instruction ("Too many sync wait commands"). Split the tail waits
# across one sync NOP per logical proc; the drain itself then needs none.
# ---------------------------------------------------------------------------
def _patch_tile_drain():
    import concourse.tile as tile
    from concourse.vector_clock import ScopedClock, VectorClock

    if getattr(tile.TileContext, "_drain_split_patch", False):
        return

    def _split_drain_and_barrier(self, tick_clock, wait_clock):
        gc = tick_clock.global_clock
        n = len(gc)
        for proc in range(n):
            t = gc[proc]
            if t > 0:
                vc = VectorClock([t if i == proc else 0 for i in range(n)])
                nop = self.nc.sync.nop(nofuse=True)
                wait_clock.add_sem_waits(nop.ins, ScopedClock({None: vc}))
        self.nc.sync.drain()
        self.nc.all_engine_barrier()
        assert self.sems is not None
        popped = self.nc._tile_sem_poison_stack.pop()
        assert popped is self._sem_poison
        self.nc.clear_and_free_semaphores(list(self.sems.allocated().values()))
        self.nc.all_engine_barrier()

    tile.TileContext._drain_and_barrier = _split_drain_and_barrier
    tile.TileContext._drain_split_patch = True


def _split_waits_bir(bir: bytes) -> bytes:
    """Walrus in this container allows only one sem wait per instruction.
    Spill extra on_wait entries onto same-engine NoOps inserted right
    before the instruction (the NX executes them in stream order)."""
    d = json.loads(bir)
    ctr = 0
    for f in d["functions"]:
        for bb in f["blocks"]:
            new = []
            for ins in bb["instructions"]:
                si = ins.get("sync_info")
                waits = si.get("on_wait", []) if si else []
                if len(waits) > 1:
                    for w in waits[:-1]:
                        ctr += 1
                        new.append(
                            {
                                "debug": ins.get("debug", 0),
                                "engine": ins["engine"],
                                "ins": [],
                                "name": f"I-wsplit-{ctr}",
                                "opcode": "NoOp",
                                "outs": [],
                                "sync_info": {"on_update": [], "on_wait": [w]},
                            }
                        )
                    si["on_wait"] = [waits[-1]]
                new.append(ins)
            bb["instructions"] = new
    return json.dumps(d).encode()


def _hook_wait_split(nc):
    orig = nc.to_json_bytes

    def patched():
        return _split_waits_bir(orig())

    nc.to_json_bytes = patched
    return nc


# ---------------------------------------------------------------------------
# Bass program (identical on all 8 cores; all per-core data arrives as
# ExternalInputs)
# ---------------------------------------------------------------------------
def _build_nc(reps=1, f32r=True, qk_bias=True, upto=3, order=None, zpipe=True, xpf=0, xsplit=4, fp8proj=True, v_bias=False):
    if order is None:
        order = 5 if ZT else 4
    import concourse.bass as bass
    import concourse.mybir as mybir
    import concourse.tile as tile

    FP = mybir.dt.float32
    FR = mybir.dt.float32r
    F8 = mybir.dt.float8e4
    DR = mybir.MatmulPerfMode.DoubleRow
    AF = mybir.ActivationFunctionType
    _patch_tile_drain()

    nc = bass.Bass(target_bir_lowering=False)

    MT = FR if f32r else FP  # dtype for every matmul operand tile

    mm = nc.tensor.matmul

    if fp8proj:
        # fp8 DoubleRow projection operands. Layouts put the contraction
        # dim D on (partition p, chunk-pair cp, subrow i): D = cp*256+i*128+p.
        x8 = nc.dram_tensor("x8", [128, 4, 2, S], F8, kind="ExternalInput")
        dx8 = nc.dram_tensor("dx8", [128, 4, 2, S], F8, kind="ExternalInput")
        wq = nc.dram_tensor("wq", [128, 4, 2, 2, 128], F8, kind="ExternalInput")
        wk = nc.dram_tensor("wk", [128, 4, 2, 2, 128], F8, kind="ExternalInput")
        wv = nc.dram_tensor("wv", [128, 4, 2, 256], F8, kind="ExternalInput")
        dwv = nc.dram_tensor("dwv", [128, 4, 2, 256], F8, kind="ExternalInput")
    else:
        xT = nc.dram_tensor("xt", [D, S], MT, kind="ExternalInput")
        wq = nc.dram_tensor("wq", [2, D, 128], MT, kind="ExternalInput")
        wk = nc.dram_tensor("wk", [2, D, 128], MT, kind="ExternalInput")
        wv = nc.dram_tensor("wv", [D, 256], MT, kind="ExternalInput")
    WOT = mybir.dt.bfloat16 if order == 5 else MT
    wo = nc.dram_tensor("wo", [2, 128, D], WOT, kind="ExternalInput")
    ident = nc.dram_tensor("ident", [128, 128], mybir.dt.bfloat16, kind="ExternalInput")
    bq = nc.dram_tensor("bq", [2, 128], FP, kind="ExternalInput")
    bk = nc.dram_tensor("bk", [2, 128], FP, kind="ExternalInput")
    bv = nc.dram_tensor("bv", [256], FP, kind="ExternalInput")
    BF = mybir.dt.bfloat16
    tri = nc.dram_tensor("tri", [128, 128], BF, kind="ExternalInput")
    sel = nc.dram_tensor("sel", [2, 128], MT, kind="ExternalInput")
    out = nc.dram_tensor("out", [S, D], FP, kind="ExternalOutput")

    with tile.TileContext(nc) as tc:
        with (
            nc.allow_low_precision(reason="tf32 (fp32r) matmul pipeline"),
            tc.tile_pool(name="consts", bufs=1) as consts,
            tc.tile_pool(name="xp", bufs=3) as xp,
            tc.tile_pool(name="qk", bufs=1) as qk,
            tc.tile_pool(name="vp", bufs=1) as vp,
            tc.tile_pool(name="zp", bufs=1) as zp,
            tc.tile_pool(name="etp", bufs=5) as etp,
            tc.tile_pool(name="bcp", bufs=2) as bcp,
            tc.tile_pool(name="rdpool", bufs=4) as rdpool,
            tc.tile_pool(name="ostp", bufs=2) as ostp,
            tc.tile_pool(name="psA", bufs=2, space="PSUM") as psA,
            tc.tile_pool(name="psB", bufs=2 if order == 5 else 4, space="PSUM") as psB,
            tc.tile_pool(name="psZTp", bufs=1, space="PSUM") as psZTp,
            tc.tile_pool(name="psDp", bufs=1, space="PSUM") as psDp,
            tc.tile_pool(name="psTp", bufs=1, space="PSUM") as psTp,
            tc.tile_pool(name="ztp", bufs=8) as ztp,
        ):
            # ---- constants ----
            # group A: needed by the first projections -- DMA'd first so the
            # startup x block isn't bandwidth-starved by cold constants
            if fp8proj:
                wq_sb = consts.tile([128, 4, 2, 2, 128], F8, tag="wq")
                nc.sync.dma_start(out=wq_sb, in_=wq[:])
                x0_t = xp.tile([128, 4, 2, 512], F8, tag="x", name="x_pre0")
                dx0_t = xp.tile([128, 4, 2, 512], F8, tag="dx", name="dx_pre0")
                nc.sync.dma_start(out=x0_t, in_=x8[:, :, :, 0:512])
                nc.sync.dma_start(out=dx0_t, in_=dx8[:, :, :, 0:512])
                wk_sb = consts.tile([128, 4, 2, 2, 128], F8, tag="wk")
                nc.sync.dma_start(out=wk_sb, in_=wk[:])
                wv_sb = consts.tile([128, 4, 2, 256], F8, tag="wv")
                nc.sync.dma_start(out=wv_sb, in_=wv[:])
                dwv_sb = consts.tile([128, 4, 2, 256], F8, tag="dwv")
                nc.sync.dma_start(out=dwv_sb, in_=dwv[:])
            else:
                xTr = xT[:].rearrange("(c p) s -> p c s", p=128)
                wq_sb = consts.tile([128, 2, 8, 128], MT, tag="wq")
                nc.sync.dma_start(
                    out=wq_sb, in_=wq[:].rearrange("a (c p) d -> p a c d", p=128)
                )
                x0_t = xp.tile([128, 8, 512], MT, tag="x", name="x_pre0")
                for dd in range(4):
                    nc.sync.dma_start(
                        out=x0_t[:, 2 * dd : 2 * dd + 2, :],
                        in_=xTr[:, 2 * dd : 2 * dd + 2, 0:512],
                    )
                wk_sb = consts.tile([128, 2, 8, 128], MT, tag="wk")
                nc.sync.dma_start(
                    out=wk_sb, in_=wk[:].rearrange("a (c p) d -> p a c d", p=128)
                )
                wv_sb = consts.tile([128, 8, 256], MT, tag="wv")
                nc.sync.dma_start(
                    out=wv_sb, in_=wv[:].rearrange("(c p) d -> p c d", p=128)
                )
            if qk_bias:
                bq_sb = consts.tile([128, 2], FP, tag="bq")
                nc.sync.dma_start(out=bq_sb, in_=bq[:].rearrange("a p -> p a"))
                bk_sb = consts.tile([128, 2], FP, tag="bk")
                nc.sync.dma_start(out=bk_sb, in_=bk[:].rearrange("a p -> p a"))
            bvbc_sb = consts.tile([128, 4, 64], FP, tag="bvbc")
            nc.sync.dma_start(
                out=bvbc_sb,
                in_=bass.AP(tensor=bv, offset=0, ap=[[0, 128], [1, 256]]),
            )
            # group B: not needed until attention / out-proj of the first
            # s-block -- emitted lazily below
            wo_sb = consts.tile([128, 2, D], WOT, tag="wo")
            ident_sb = consts.tile([128, 128], BF, tag="ident")
            tri_sb = consts.tile([128, 128], BF, tag="tri")
            sel_sb = consts.tile([1, 2, 128], MT, tag="sel")

            def emit_const_group_b():
                nc.sync.dma_start(out=tri_sb, in_=tri[:])
                if order == 5:
                    nc.sync.dma_start(out=ident_sb, in_=ident[:])
                nc.sync.dma_start(
                    out=sel_sb,
                    in_=bass.AP(
                        tensor=sel, offset=0, ap=[[256, 1], [128, 2], [1, 128]]
                    ),
                )
                nc.sync.dma_start(
                    out=wo_sb, in_=wo[:].rearrange("a p d -> p a d")
                )

            def emit_x(sb, x_pre=None):
                if x_pre is not None:
                    return x_pre
                if fp8proj:
                    x_t = xp.tile([128, 4, 2, 512], F8, tag="x", name=f"x_{sb}")
                    dx_t = xp.tile([128, 4, 2, 512], F8, tag="dx", name=f"dx_{sb}")
                    for dd in range(2):
                        nc.sync.dma_start(
                            out=x_t[:, 2 * dd : 2 * dd + 2, :, :],
                            in_=x8[:, 2 * dd : 2 * dd + 2, :, sb * 512 : (sb + 1) * 512],
                        )
                        nc.sync.dma_start(
                            out=dx_t[:, 2 * dd : 2 * dd + 2, :, :],
                            in_=dx8[:, 2 * dd : 2 * dd + 2, :, sb * 512 : (sb + 1) * 512],
                        )
                    return (x_t, dx_t)
                x_t = xp.tile([128, 8, 512], MT, tag="x", name=f"x_{sb}")
                w = 8 // xsplit
                for dd in range(xsplit):
                    nc.sync.dma_start(
                        out=x_t[:, w * dd : w * dd + w, :],
                        in_=xTr[
                            :, w * dd : w * dd + w, sb * 512 : (sb + 1) * 512
                        ],
                    )
                return x_t

            # scale folding for the fp8 path: weights host-quantized at x64,
            # score 1/sqrt(HD)=1/8 folded into the Q copy
            SQ, SK, SV = 2.0**-9, 2.0**-6, 2.0**-6

            def emit_qk(sb, x_t, qt_sb, kt_sb, pairs=(0, 1)):
                if fp8proj:
                    x_t = x_t[0]
                for pair in pairs:
                    psQ = psA.tile([128, 2, 512], FP, tag="A", name=f"psQ_{sb}_{pair}")
                    if fp8proj:
                        for c in range(4):
                            mm(
                                psQ[:, 0, :],
                                wq_sb[:, c, :, pair, :],
                                x_t[:, c, :, :],
                                start=(c == 0),
                                stop=(c == 3),
                                perf_mode=DR,
                            )
                        for c in range(4):
                            mm(
                                psQ[:, 1, :],
                                wk_sb[:, c, :, pair, :],
                                x_t[:, c, :, :],
                                start=(c == 0),
                                stop=(c == 3),
                                perf_mode=DR,
                            )
                    else:
                        for c in range(8):
                            mm(
                                psQ[:, 0, :],
                                wq_sb[:, pair, c, :],
                                x_t[:, c, :],
                                start=(c == 0),
                                stop=(c == 7),
                            )
                        for c in range(8):
                            mm(
                                psQ[:, 1, :],
                                wk_sb[:, pair, c, :],
                                x_t[:, c, :],
                                start=(c == 0),
                                stop=(c == 7),
                            )
                    qt_dst = qt_sb[:, pair, sb * 512 : (sb + 1) * 512]
                    kt_dst = kt_sb[:, pair, sb * 512 : (sb + 1) * 512]
                    sq = SQ if fp8proj else 1.0
                    sk = SK if fp8proj else 1.0
                    if qk_bias:
                        nc.scalar.activation(
                            qt_dst, psQ[:, 0, :], AF.Identity,
                            bias=bq_sb[:, pair : pair + 1], scale=sq,
                        )
                        nc.scalar.activation(
                            kt_dst, psQ[:, 1, :], AF.Identity,
                            bias=bk_sb[:, pair : pair + 1], scale=sk,
                        )
                    elif fp8proj:
                        if pair == 0:
                            nc.vector.tensor_scalar_mul(qt_dst, psQ[:, 0, :], sq)
                            nc.vector.tensor_scalar_mul(kt_dst, psQ[:, 1, :], sk)
                        else:
                            nc.scalar.activation(qt_dst, psQ[:, 0, :], AF.Copy, scale=sq)
                            nc.scalar.activation(kt_dst, psQ[:, 1, :], AF.Copy, scale=sk)
                    elif pair == 0:
                        nc.vector.tensor_copy(qt_dst, psQ[:, 0, :])
                        nc.vector.tensor_copy(kt_dst, psQ[:, 1, :])
                    else:
                        nc.scalar.activation(qt_dst, psQ[:, 0, :], AF.Copy)
                        nc.scalar.activation(kt_dst, psQ[:, 1, :], AF.Copy)

            def emit_v(sb, x_t, v_sb):
                if fp8proj:
                    x_t, dx_t = x_t
                for stl in range(4):
                    st = sb * 4 + stl
                    psV = (psA if order == 5 else psB).tile(
                        [128, 256], FP, tag="A" if order == 5 else "ZB",
                        name=f"psV_{st}",
                    )
                    if fp8proj:
                        ops = [(x_t, wv_sb), (dx_t, wv_sb), (x_t, dwv_sb)]
                        for t, (xx, ww) in enumerate(ops):
                            for c in range(4):
                                mm(
                                    psV,
                                    xx[:, c, :, stl * 128 : (stl + 1) * 128],
                                    ww[:, c, :, :],
                                    start=(t == 0 and c == 0),
                                    stop=(t == 2 and c == 3),
                                    perf_mode=DR,
                                )
                        nc.vector.tensor_scalar_mul(
                            v_sb[:, st, :, 0:64],
                            psV.rearrange("p (h d) -> p h d", h=4),
                            SV,
                        )
                        if v_bias:
                            nc.vector.tensor_add(
                                v_sb[:, st, :, 0:64],
                                v_sb[:, st, :, 0:64],
                                bvbc_sb,
                            )
                    else:
                        for c in range(8):
                            mm(
                                psV,
                                x_t[:, c, stl * 128 : (stl + 1) * 128],
                                wv_sb[:, c, :],
                                start=(c == 0),
                                stop=(c == 7),
                            )
                        nc.vector.tensor_add(
                            v_sb[:, st, :, 0:64],
                            psV.rearrange("p (h d) -> p h d", h=4),
                            bvbc_sb,
                        )

            zt_tiles = {}

            def emit_attn_zt(pair, Qb, qt_sb, kt_sb, v_sb, psZTs,
                             filler=None, post_kt=None):
                """Scores/exp as before, but z runs in natural [q, d]
                orientation: e_t column-slices are the stationary, V'_ph
                [128, 65] the moving, accumulating psZT[qcl][128q, ph, 65]
                over k-tiles (col 64 = softmax denominator)."""
                q0, q1 = Qb * 512, (Qb + 1) * 512
                ktmax = 4 * (Qb + 1)
                pending_z = None
                for kt in range(ktmax):
                    diag = kt >= 4 * Qb
                    r = (kt - 4 * Qb) * 128 if diag else 0
                    psS = psA.tile(
                        [128, 2, 512], FP, tag="A", name=f"psSz_{pair}_{Qb}_{kt}"
                    )
                    for hh in range(2):
                        po = hh * 64
                        mm(
                            psS[:, hh, r:512],
                            kt_sb[po : po + 64, pair, kt * 128 : (kt + 1) * 128],
                            qt_sb[po : po + 64, pair, q0 + r : q1],
                            start=True,
                            stop=True,
                        )
                    e_t = etp.tile(
                        [128, 2, 512], BF, tag="et", name=f"etz_{pair}_{Qb}_{kt}"
                    )
                    nc.scalar.activation(
                        e_t[:, :, r:512], psS[:, :, r:512], AF.Exp
                    )
                    if diag:
                        for hh in range(2):
                            nc.vector.tensor_mul(
                                e_t[:, hh, r : r + 128],
                                e_t[:, hh, r : r + 128],
                                tri_sb,
                            )
                    psZT, psD = psZTs

                    def emit_zd(kt, e_t, qcl0):
                        for hh in range(2):
                            ph = 2 * pair + hh
                            for qcl in range(qcl0, 4):
                                e_sl = e_t[:, hh, qcl * 128 : (qcl + 1) * 128]
                                # start_tensor_calc zeroes the WHOLE 2KB
                                # bank, so only the chronologically-first
                                # writer of each bank may set it; all other
                                # accumulators in the bank start=False onto
                                # the pending-zero bytes
                                mm(
                                    psZT[:, qcl, ph, :],
                                    e_sl,
                                    v_sb[:, kt, ph, 0:64],
                                    start=(kt == 0 and ph == 0
                                           and qcl in (0, 2)),
                                    stop=(kt == 4 * Qb + qcl),
                                    skip_group_check=True,
                                )
                                mm(
                                    psD[:, qcl, ph, :],
                                    e_sl,
                                    v_sb[:, kt, ph, 64:65],
                                    start=(kt == 0 and ph == 0 and qcl == 0),
                                    stop=(kt == 4 * Qb + qcl),
                                    skip_group_check=True,
                                )

                    qcl0 = (kt - 4 * Qb) if diag else 0
                    # z one k-tile behind: its exp finished during this
                    # slot's score matmuls, so PE never waits on it
                    if pending_z is not None:
                        emit_zd(*pending_z)
                        if post_kt is not None:
                            post_kt(pending_z[0])
                    pending_z = (kt, e_t, qcl0)
                    if filler is not None:
                        filler()
                if pending_z is not None:
                    emit_zd(*pending_z)
                    if post_kt is not None:
                        post_kt(pending_z[0])

            def emit_norm_zt(Qb, qcl, psZTs):
                """Per-partition normalization (partitions = q), then PE
                transpose to z^T for the output projection."""
                psZT, psD = psZTs
                st = 4 * Qb + qcl
                rd4 = rdpool.tile([128, 4, 1], FP, tag="rd4", name=f"rd4_{st}")
                nc.vector.reciprocal(rd4, psD[:, qcl, :, :])
                znat = bcp.tile([128, 4, 64], BF, tag="znat", name=f"znat_{st}")
                for ph in range(4):
                    nc.vector.tensor_scalar_mul(
                        znat[:, ph, :], psZT[:, qcl, ph, :], rd4[:, ph, :]
                    )
                zt = ztp.tile([128, 2, 128], BF, tag="ztq", name=f"zt_{st}")
                for ch in range(2):
                    # separate grabs: the shared 1-bank pool's rotation makes
                    # ch1's bank-zeroing start wait for ch0's copy to read
                    psT = psTp.tile(
                        [128, 128], BF, tag="T", name=f"psT_{st}_{ch}"
                    )
                    nc.tensor.transpose(
                        psT, znat[:, 2 * ch : 2 * ch + 2, :], ident_sb
                    )
                    nc.vector.tensor_copy(zt[:, ch, :], psT)
                zt_tiles[st] = zt

            def emit_out_unit_zt(st, db):
                """One 512-column half of the output projection."""
                if db == 0:
                    ost_tiles[st] = ostp.tile(
                        [128, D], FP, tag="ost", name=f"ostz_{st}"
                    )
                ost_t = ost_tiles[st]
                zt = zt_tiles[st]
                psO = psTp.tile([128, 512], FP, tag="T", name=f"psOz_{st}_{db}")
                for pr in range(2):
                    mm(
                        psO,
                        zt[:, pr, :],
                        wo_sb[:, pr, db * 512 : (db + 1) * 512],
                        start=(pr == 0),
                        stop=(pr == 1),
                    )
                if db % 2 == 0:
                    nc.vector.tensor_copy(
                        ost_t[:, db * 512 : (db + 1) * 512], psO
                    )
                else:
                    nc.scalar.activation(
                        ost_t[:, db * 512 : (db + 1) * 512], psO, AF.Copy
                    )
                if db == 1:
                    nc.sync.dma_start(
                        out=out[st * 128 : (st + 1) * 128, :], in_=ost_t
                    )
                    del zt_tiles[st]
                    del ost_tiles[st]

            def emit_attn_both(Qb, qt_sb, kt_sb, v_sb, znp):
                """Both head pairs' attention for one q-block, kt loops
                interleaved: two independent score->exp->z chains keep PE
                fed while either waits on ScalarE."""
                q0, q1 = Qb * 512, (Qb + 1) * 512
                ktmax = 4 * (Qb + 1)
                psZs = {}
                for pair in range(2):
                    for hh in range(2):
                        psZs[(pair, hh)] = psB.tile(
                            [65, 512], FP, tag="ZB",
                            name=f"psZ_{pair}_{Qb}_{hh}",
                        )
                for kt in range(ktmax):
                    diag = kt >= 4 * Qb
                    r = (kt - 4 * Qb) * 128 if diag else 0
                    for pair in range(2):
                        psS = psA.tile(
                            [128, 2, 512], FP, tag="A",
                            name=f"psS_{pair}_{Qb}_{kt}",
                        )
                        for hh in range(2):
                            po = hh * 64
                            mm(
                                psS[:, hh, r:512],
                                kt_sb[po : po + 64, pair, kt * 128 : (kt + 1) * 128],
                                qt_sb[po : po + 64, pair, q0 + r : q1],
                                start=True,
                                stop=True,
                            )
                        e_t = etp.tile(
                            [128, 2, 512], BF, tag="et",
                            name=f"et_{pair}_{Qb}_{kt}",
                        )
                        nc.scalar.activation(
                            e_t[:, :, r:512], psS[:, :, r:512], AF.Exp
                        )
                        if diag:
                            for hh in range(2):
                                nc.vector.tensor_mul(
                                    e_t[:, hh, r : r + 128],
                                    e_t[:, hh, r : r + 128],
                                    tri_sb,
                                )
                        for hh in range(2):
                            mm(
                                psZs[(pair, hh)][:, r:512],
                                v_sb[:, kt, 2 * pair + hh, :],
                                e_t[:, hh, r:512],
                                start=(kt == 0),
                                stop=(kt == ktmax - 1),
                            )
                for pair in range(2):
                    rds = []
                    for hh in range(2):
                        rd_h = rdpool.tile(
                            [1, 512], MT, tag="rd", name=f"rd_{pair}_{Qb}_{hh}"
                        )
                        rds.append(rd_h)
                        nc.vector.reciprocal(rd_h, psZs[(pair, hh)][64:65, :])
                    bc = psA.tile([128, 512], FP, tag="A", name=f"bc_{pair}_{Qb}")
                    mm(bc, sel_sb[:, 0, :], rds[0], start=True, stop=False)
                    mm(bc, sel_sb[:, 1, :], rds[1], start=False, stop=True)
                    bcs = bcp.tile(
                        [128, 512], FP, tag="bcs", name=f"bcs_{pair}_{Qb}"
                    )
                    nc.vector.tensor_copy(bcs, bc)
                    nc.vector.tensor_mul(
                        znp[0:64, pair, Qb, :],
                        psZs[(pair, 0)][0:64, :],
                        bcs[0:64, :],
                    )
                    zc = bcp.tile([128, 512], FP, tag="zc", name=f"zc_{pair}_{Qb}")
                    nc.vector.tensor_copy(zc[64:128, :], psZs[(pair, 1)][0:64, :])
                    nc.vector.tensor_mul(
                        znp[64:128, pair, Qb, :],
                        zc[64:128, :],
                        bcs[64:128, :],
                    )

            def emit_attn(pair, Qb, qt_sb, kt_sb, v_sb, znp, filler=None):
                """Attention for one head pair and one 512-wide q-block."""
                q0, q1 = Qb * 512, (Qb + 1) * 512
                ktmax = 4 * (Qb + 1)
                psZs = []
                for hh in range(2):
                    psZ_h = psB.tile(
                        [65, 512], FP, tag="ZB", name=f"psZ_{pair}_{Qb}_{hh}"
                    )
                    psZs.append(psZ_h)
                def emit_z(kt, e_t, r):
                    for hh in range(2):
                        mm(
                            psZs[hh][:, r:512],
                            v_sb[:, kt, 2 * pair + hh, :],
                            e_t[:, hh, r:512],
                            start=(kt == 0),
                            stop=(kt == ktmax - 1),
                        )

                pending = None  # (kt, e_t, r) -- z emitted one kt behind
                for kt in range(ktmax):
                    # diagonal k-tiles: q-columns < r are fully masked, so
                    # scores/exp/z are all computed on [r:512] only (bf16
                    # matmuls have no minimum-N rate constraint)
                    diag = kt >= 4 * Qb
                    r = (kt - 4 * Qb) * 128 if diag else 0
                    # both heads' scores in one 2-bank PSUM tile; the two
                    # K=64 matmuls hit disjoint PE row groups and overlap
                    psS = psA.tile(
                        [128, 2, 512], FP, tag="A", name=f"psS_{pair}_{Qb}_{kt}"
                    )
                    for hh in range(2):
                        po = hh * 64
                        mm(
                            psS[:, hh, r:512],
                            kt_sb[po : po + 64, pair, kt * 128 : (kt + 1) * 128],
                            qt_sb[po : po + 64, pair, q0 + r : q1],
                            start=True,
                            stop=True,
                        )
                    e_t = etp.tile(
                        [128, 2, 512], BF, tag="et", name=f"et_{pair}_{Qb}_{kt}"
                    )
                    nc.scalar.activation(
                        e_t[:, :, r:512], psS[:, :, r:512], AF.Exp
                    )
                    if diag:
                        # causal 0/1 mask: only the 128-wide diagonal band
                        # needs it; columns past the band are unmasked
                        for hh in range(2):
                            nc.vector.tensor_mul(
                                e_t[:, hh, r : r + 128],
                                e_t[:, hh, r : r + 128],
                                tri_sb,
                            )
                    if not zpipe:
                        emit_z(kt, e_t, r)
                    else:
                        if pending is not None:
                            emit_z(*pending)
                        pending = (kt, e_t, r)
                    if filler is not None:
                        filler()
                if zpipe:
                    emit_z(*pending)
                rds = []
                for hh in range(2):
                    rd_h = rdpool.tile(
                        [1, 512], MT, tag="rd", name=f"rd_{pair}_{Qb}_{hh}"
                    )
                    rds.append(rd_h)
                    nc.vector.reciprocal(rd_h, psZs[hh][64:65, :])
                # broadcast 1/denom of both heads to a stacked [128, 512]
                # tile via two K=1 matmuls against selector rows
                bc = psB.tile([128, 512], FP, tag="ZB", name=f"bc_{pair}_{Qb}")
                mm(bc, sel_sb[:, 0, :], rds[0], start=True, stop=False)
                mm(bc, sel_sb[:, 1, :], rds[1], start=False, stop=True)
                bcs = bcp.tile([128, 512], FP, tag="bcs", name=f"bcs_{pair}_{Qb}")
                nc.vector.tensor_copy(bcs, bc)
                # hh=0: partitions already 0..63 everywhere
                nc.vector.tensor_mul(
                    znp[0:64, pair, Qb, :],
                    psZs[0][0:64, :],
                    bcs[0:64, :],
                )
                # hh=1: single-src shift-copy 0..63 -> 64..127, then mul
                zc = bcp.tile([128, 512], FP, tag="zc", name=f"zc_{pair}_{Qb}")
                nc.vector.tensor_copy(zc[64:128, :], psZs[1][0:64, :])
                nc.vector.tensor_mul(
                    znp[64:128, pair, Qb, :],
                    zc[64:128, :],
                    bcs[64:128, :],
                )

            ost_tiles = {}

            def emit_out_unit(st, Db, znp):
                """One Db half of the output projection for one s-tile."""
                Qb, soff = st // 4, (st % 4) * 128
                if Db == 0:
                    ost_tiles[st] = ostp.tile(
                        [128, D], FP, tag="ost", name=f"ost_{st}"
                    )
                ost_t = ost_tiles[st]
                if True:
                    psO = psB.tile(
                        [128, 512], FP, tag="ZB", name=f"psO_{st}_{Db}"
                    )
                    for pair in range(2):
                        mm(
                            psO,
                            znp[:, pair, Qb, soff : soff + 128],
                            wo_sb[:, pair, Db * 512 : (Db + 1) * 512],
                            start=(pair == 0),
                            stop=(pair == 1),
                        )
                    if Db == 0:
                        nc.vector.tensor_copy(
                            ost_t[:, Db * 512 : (Db + 1) * 512], psO
                        )
                    else:
                        nc.scalar.activation(
                            ost_t[:, Db * 512 : (Db + 1) * 512], psO, AF.Copy
                        )
                if Db == 1:
                    # issue from the DVE sequencer: the store's wait target
                    # is the DVE copy that just ran there, so it can't
                    # head-of-line block the SP stream that issues x loads
                    nc.sync.dma_start(
                        out=out[st * 128 : (st + 1) * 128, :], in_=ost_t
                    )
                    del ost_tiles[st]

            def emit_out(st, znp):
                for Db in range(2):
                    emit_out_unit(st, Db, znp)

            for _rep in range(reps):
                qt_sb = qk.tile([128, 2, S], BF, tag="qt")
                kt_sb = qk.tile([128, 2, S], BF, tag="kt")
                v_sb = vp.tile([128, 16, 4, 65], BF, tag="v")
                znp = zp.tile([128, 2, 4, 512], MT, tag="zn")
                # ones column of V' (written once; proj fills the rest)
                nc.vector.memset(v_sb[:, :, :, 64:65], 1.0)

                # interleaved emission: attention for q-block Qb only needs
                # projections of s-blocks <= Qb, so proj(sb) / attn(Qb=sb) /
                # out-proj(Qb=sb) alternate -- projection PE work fills the
                # gaps while ScalarE grinds through the exps
                x_tiles = {}
                x_pre0 = (x0_t, dx0_t) if fp8proj else x0_t
                for sb in range(4):
                    if sb not in x_tiles:
                        x_tiles[sb] = emit_x(
                            sb, x_pre=x_pre0 if (_rep == 0 and sb == 0) else None
                        )
                    x_t = x_tiles[sb]
                    # prefetch x blocks ahead
                    for ahead in range(1, xpf + 1):
                        if sb + ahead <= 3 and sb + ahead not in x_tiles:
                            x_tiles[sb + ahead] = emit_x(sb + ahead)
                    if order == 5:
                        # transposed-z: norm+transpose emitted inside pair1's
                        # kt loop as soon as each q-chunk's accumulation
                        # finishes; out-proj quarters of the PREVIOUS s-block
                        # drain one per kt slot as PE filler
                        emit_qk(sb, x_t, qt_sb, kt_sb)
                        emit_v(sb, x_t, v_sb)
                        if _rep == 0 and sb == 0:
                            emit_const_group_b()
                        psZTs = (
                            psZTp.tile(
                                [128, 4, 4, 64], FP, tag="zt",
                                name=f"psZT_{sb}",
                            ),
                            psDp.tile(
                                [128, 4, 4, 1], FP, tag="zd",
                                name=f"psD_{sb}",
                            ),
                        )
                        pend = (
                            [(st, db) for st in range(4 * sb - 4, 4 * sb)
                             for db in range(2)]
                            if sb > 0 else []
                        )
                        slots = [2 * 4 * (sb + 1)]

                        def filler(pend=pend, slots=slots):
                            slots[0] -= 1
                            if not pend:
                                return
                            k = max(1, -(-len(pend) // max(1, slots[0] + 1)))
                            for _ in range(min(k, len(pend))):
                                st, db = pend.pop(0)
                                emit_out_unit_zt(st, db)

                        def post_kt(kt, sb=sb, psZTs=psZTs):
                            if kt >= 4 * sb:
                                emit_norm_zt(sb, kt - 4 * sb, psZTs)

                        emit_attn_zt(0, sb, qt_sb, kt_sb, v_sb, psZTs, filler)
                        emit_attn_zt(1, sb, qt_sb, kt_sb, v_sb, psZTs, filler,
                                     post_kt)
                        for st, db in pend:
                            emit_out_unit_zt(st, db)
                        if sb == 3:
                            for st in range(12, 16):
                                for db in range(2):
                                    emit_out_unit_zt(st, db)
                    elif order == 4:
                        # out-proj of the previous s-block drained one Db-unit
                        # at a time between attention kt slots: PE filler for
                        # the ACT-paced exp stream
                        emit_qk(sb, x_t, qt_sb, kt_sb)
                        emit_v(sb, x_t, v_sb)
                        if _rep == 0 and sb == 0:
                            emit_const_group_b()
                        pend = (
                            [(st, Db) for st in range(4 * sb - 4, 4 * sb)
                             for Db in range(2)]
                            if sb > 0 else []
                        )
                        slots = [2 * 4 * (sb + 1)]

                        def filler(pend=pend, slots=slots):
                            slots[0] -= 1
                            if not pend:
                                return
                            k = max(1, -(-len(pend) // max(1, slots[0] + 1)))
                            for _ in range(min(k, len(pend))):
                                st, Db = pend.pop(0)
                                emit_out_unit(st, Db, znp)

                        if upto >= 2:
                            emit_attn(0, sb, qt_sb, kt_sb, v_sb, znp, filler)
                            emit_attn(1, sb, qt_sb, kt_sb, v_sb, znp, filler)
                        for st, Db in pend:
                            emit_out_unit(st, Db, znp)
                        if sb == 3:
                            for st in range(12, 16):
                                emit_out(st, znp)
                    elif order == 3:
                        emit_qk(sb, x_t, qt_sb, kt_sb)
                        emit_v(sb, x_t, v_sb)
                        if _rep == 0 and sb == 0:
                            emit_const_group_b()
                        if upto >= 2:
                            emit_attn_both(sb, qt_sb, kt_sb, v_sb, znp)
                    elif order == 0:
                        emit_qk(sb, x_t, qt_sb, kt_sb)
                        emit_v(sb, x_t, v_sb)
                        if _rep == 0 and sb == 0:
                            emit_const_group_b()
                        if upto >= 2:
                            for pair in range(2):
                                emit_attn(pair, sb, qt_sb, kt_sb, v_sb, znp)
                    elif order == 2:
                        # out-proj of the previous s-block emitted between
                        # the two attention passes as mid-segment PE filler
                        emit_qk(sb, x_t, qt_sb, kt_sb)
                        emit_v(sb, x_t, v_sb)
                        if _rep == 0 and sb == 0:
                            emit_const_group_b()
                        if upto >= 2:
                            emit_attn(0, sb, qt_sb, kt_sb, v_sb, znp)
                        if upto >= 3 and sb > 0:
                            for st in range(4 * sb - 4, 4 * sb):
                                emit_out(st, znp)
                        if upto >= 2:
                            emit_attn(1, sb, qt_sb, kt_sb, v_sb, znp)
                    else:
                        # pair-1 projections emitted between the two
                        # attention passes: PE fills attention(pair0)'s
                        # ScalarE-bound stretch with projection matmuls
                        emit_qk(sb, x_t, qt_sb, kt_sb, pairs=(0,))
                        emit_v(sb, x_t, v_sb)
                        if _rep == 0 and sb == 0:
                            emit_const_group_b()
                        if upto >= 2:
                            emit_attn(0, sb, qt_sb, kt_sb, v_sb, znp)
                        emit_qk(sb, x_t, qt_sb, kt_sb, pairs=(1,))
                        if upto >= 2:
                            emit_attn(1, sb, qt_sb, kt_sb, v_sb, znp)
                    if upto >= 3 and (order not in (2, 4, 5) or sb == 3):
                        if order in (4, 5):
                            continue
                        for st in range(4 * sb, 4 * sb + 4):
                            emit_out(st, znp)

    return _hook_wait_split(nc)



# ---------------------------------------------------------------------------
# Persistent PJRT runner (mirrors run_bass_via_pjrt, but keeps the jitted
# callable so repeated kernel() calls don't recompile)
# ---------------------------------------------------------------------------
class _Runner:
    def __init__(self, nc):
        import jax
        import jax.numpy as jnp  # noqa: F401
        import numpy as _np
        from jax.experimental.shard_map import shard_map
        from jax.sharding import Mesh, PartitionSpec
        import concourse.mybir as mybir
        from concourse.bass2jax import (
            _bass_exec_p,
            install_neuronx_cc_hook,
            partition_id_tensor,
        )

        install_neuronx_cc_hook()
        self.jax = jax
        pname = nc.partition_id_tensor.name if nc.partition_id_tensor else None
        in_names, out_names, out_avals, zero_outs = [], [], [], []
        for alloc in nc.m.functions[0].allocations:
            if not isinstance(alloc, mybir.MemoryLocationSet):
                continue
            name = alloc.memorylocations[0].name
            if alloc.kind == "ExternalInput":
                if name == pname:
                    continue
                in_names.append(name)
            elif alloc.kind == "ExternalOutput":
                shape = tuple(alloc.tensor_shape)
                dtype = mybir.dt.np(alloc.dtype)
                out_names.append(name)
                out_avals.append(jax.core.ShapedArray(shape, dtype))
                zero_outs.append(_np.zeros(shape, dtype))
        self.in_names, self.out_names = list(in_names), list(out_names)
        self.out_avals, self.zero_outs = out_avals, zero_outs
        n_params, n_outs = len(in_names), len(out_names)
        self.n_params = n_params
        all_names = in_names + out_names
        if pname is not None:
            all_names = all_names + [pname]

        def _body(*args):
            operands = list(args)
            if pname is not None:
                operands.append(partition_id_tensor())
            outs = _bass_exec_p.bind(
                *operands,
                out_avals=tuple(out_avals),
                in_names=tuple(all_names),
                out_names=tuple(out_names),
                lowering_input_output_aliases=(),
                sim_require_finite=True,
                sim_require_nnan=True,
                nc=nc,
            )
            return tuple(outs)

        devices = jax.devices()[:NCORES]
        mesh = Mesh(np.asarray(devices), ("core",))
        in_specs = (PartitionSpec("core"),) * (n_params + n_outs)
        out_specs = (PartitionSpec("core"),) * n_outs
        self.fn = jax.jit(
            shard_map(
                _body,
                mesh=mesh,
                in_specs=in_specs,
                out_specs=out_specs,
                check_rep=False,
            ),
            donate_argnums=tuple(range(n_params, n_params + n_outs)),
            keep_unused=True,
        )

    def device_put_inputs(self, concat_in):
        return [self.jax.device_put(a) for a in concat_in]

    def time_exec(self, dev_in, iters=8):
        """Repeat execution with device-resident inputs; the previous call's
        (donated, fully-overwritten) outputs serve as the next call's output
        buffers, so nothing moves over the axon tunnel."""
        import time as _time

        zeros = [
            np.zeros((NCORES * z.shape[0], *z.shape[1:]), z.dtype)
            for z in self.zero_outs
        ]
        r = self.fn(*dev_in, *zeros)
        self.jax.block_until_ready(r)
        times = []
        for _ in range(iters):
            t0 = _time.perf_counter()
            r = self.fn(*dev_in, *r)
            self.jax.block_until_ready(r)
            times.append(_time.perf_counter() - t0)
        return times

    def concat_inputs(self, in_maps):
        return [
            np.concatenate([in_maps[c][n] for c in range(NCORES)], axis=0)
            for n in self.in_names
        ]

    def run_concat(self, concat_in):
        zeros = [
            np.zeros((NCORES * z.shape[0], *z.shape[1:]), z.dtype)
            for z in self.zero_outs
        ]
        outs = self.fn(*concat_in, *zeros)
        outs = [np.asarray(o) for o in outs]
        return outs

    def run(self, in_maps):
        outs = self.run_concat(self.concat_inputs(in_maps))
        per_core = []
        for c in range(NCORES):
            m = {}
            for i, n in enumerate(self.out_names):
                shp = self.out_avals[i].shape
                m[n] = outs[i].reshape(NCORES, *shp)[c]
            per_core.append(m)
        return per_core


def _round_tf32(a):
    """Round fp32 -> TF32 (10-bit mantissa, RNE) so device-side fp32r
    consumers see pre-rounded values."""
    u = np.ascontiguousarray(a, dtype=np.float32).view(np.uint32)
    r = (u + np.uint32(0x1000) + ((u >> np.uint32(13)) & np.uint32(1))) & np.uint32(0xFFFFE000)
    return r.view(np.float32)


def _make_tri():
    """Within-band causal 0/1 mask: keep k-row p for band column j iff p<=j."""
    import ml_dtypes

    p = np.arange(128)[:, None]
    j = np.arange(128)[None, :]
    return (p <= j).astype(ml_dtypes.bfloat16)


def _q8(a):
    import ml_dtypes

    return np.asarray(a, dtype=np.float32).astype(ml_dtypes.float8_e4m3)


def _pack_d(a):
    """[D, ...] -> [128, 4, 2, ...] with D = cp*256 + i*128 + p."""
    rest = a.shape[1:]
    return np.ascontiguousarray(
        a.reshape(4, 2, 128, *rest).transpose(2, 0, 1, *range(3, 3 + len(rest)))
    )


def _prep_core_inputs(inputs, fp8proj=True):
    """Shard + repack the full problem inputs into per-core input maps."""
    x = np.asarray(inputs["normalized_resid_pre"], dtype=np.float32)
    W_Q = np.asarray(inputs["W_Q"], dtype=np.float32)
    W_K = np.asarray(inputs["W_K"], dtype=np.float32)
    W_V = np.asarray(inputs["W_V"], dtype=np.float32)
    W_O = np.asarray(inputs["W_O"], dtype=np.float32)
    b_Q = np.asarray(inputs["b_Q"], dtype=np.float32)
    b_K = np.asarray(inputs["b_K"], dtype=np.float32)
    b_V = np.asarray(inputs["b_V"], dtype=np.float32)

    scale = np.float32(1.0 / np.sqrt(HD))
    SW = np.float32(64.0)  # fp8 weight pre-scale (power of 2)
    tri = _make_tri()
    sel = np.zeros((2, 128), dtype=np.float32)
    sel[0, 0:64] = 1.0
    sel[1, 64:128] = 1.0

    in_maps = []
    for c in range(NCORES):
        b, g = c // 4, c % 4
        hs = [4 * g + i for i in range(HPC)]
        wo_p = np.zeros((2, 128, D), dtype=np.float32)
        bq_p = np.zeros((2, 128), dtype=np.float32)
        bk_p = np.zeros((2, 128), dtype=np.float32)
        wq_p = np.zeros((2, D, 128), dtype=np.float32)
        wk_p = np.zeros((2, D, 128), dtype=np.float32)
        for pr in range(2):
            h0, h1 = hs[2 * pr], hs[2 * pr + 1]
            qsc = 1.0 if fp8proj else scale
            wq_p[pr, :, 0:64] = W_Q[h0] * qsc
            wq_p[pr, :, 64:128] = W_Q[h1] * qsc
            wk_p[pr, :, 0:64] = W_K[h0]
            wk_p[pr, :, 64:128] = W_K[h1]
            wo_p[pr, 0:64, :] = W_O[h0]
            wo_p[pr, 64:128, :] = W_O[h1]
            bq_p[pr, 0:64] = b_Q[h0] * scale
            bq_p[pr, 64:128] = b_Q[h1] * scale
            bk_p[pr, 0:64] = b_K[h0]
            bk_p[pr, 64:128] = b_K[h1]
        wv_p = np.concatenate([W_V[h] for h in hs], axis=1)  # [D, 256]
        bv_p = np.concatenate([b_V[h] for h in hs], axis=0)  # [256]
        import ml_dtypes

        m = {
            "wo": wo_p.astype(ml_dtypes.bfloat16) if ZT else _round_tf32(wo_p),
            "ident": np.eye(128, dtype=ml_dtypes.bfloat16),
            "bq": bq_p,
            "bk": bk_p,
            "bv": np.ascontiguousarray(bv_p),
            "tri": tri,
            "sel": sel,
        }
        if fp8proj:
            xT = np.ascontiguousarray(x[b].T)  # [D, S]
            x8 = _q8(xT)
            dx8 = _q8(xT - x8.astype(np.float32))
            wq8 = _q8(wq_p.transpose(1, 0, 2) * SW)  # [D, 2, 128]
            wk8 = _q8(wk_p.transpose(1, 0, 2) * SW)
            wv16 = wv_p * SW
            wv8 = _q8(wv16)
            dwv8 = _q8(wv16 - wv8.astype(np.float32))
            m.update(
                x8=_pack_d(x8),
                dx8=_pack_d(dx8),
                wq=_pack_d(wq8),
                wk=_pack_d(wk8),
                wv=_pack_d(wv8),
                dwv=_pack_d(dwv8),
            )
        else:
            m.update(
                xt=_round_tf32(np.ascontiguousarray(x[b].T)),
                wq=_round_tf32(wq_p),
                wk=_round_tf32(wk_p),
                wv=np.ascontiguousarray(_round_tf32(wv_p)),
            )
        in_maps.append(m)
    return in_maps


def _get_state(qk_bias=True, v_bias=False):
    key = (qk_bias, v_bias)
    if key not in _STATE:
        _STATE[key] = _Runner(_build_nc(qk_bias=qk_bias, v_bias=v_bias))
    return _STATE[key]


def kernel(**inputs):
    need_qk_bias = bool(
        np.any(np.asarray(inputs["b_Q"])) or np.any(np.asarray(inputs["b_K"]))
    )
    need_v_bias = bool(np.any(np.asarray(inputs["b_V"])))
    st = _get_state(qk_bias=need_qk_bias, v_bias=need_v_bias)
    in_maps = _prep_core_inputs(inputs)
    per_core = st.run(in_maps)
    b_O = np.asarray(inputs["b_O"], dtype=np.float32)
    out = np.zeros((B, S, D), dtype=np.float32)
    for c in range(NCORES):
        out[c // 4] += per_core[c]["out"]
    out += b_O[None, None, :]
    return out

